# revision 49
# baseline (speedup 1.0000x reference)
"""Trainium2 Bass kernel for nn_Jointer: per-sample masked cosine-similarity.

out[b] = relu(l2norm(source[b]) @ l2norm(target[b]).T) * (mask_src[b] outer mask_tar[b])

Sharding: data-parallel over batch B=8 -> one sample per NeuronCore.

Ragged-sequence strategy: ~half the tokens are masked out.  The host
compacts valid tokens, l2-normalizes, transposes to [D, token] fp16 and
packs source+target into one input tensor.  The device computes a
1024x1024 valid-x-valid core (exactly 8x2 full 512-wide matmul tiles)
and quantizes the relu'd similarities to uint8 (x250) on the way out of
PSUM, so the output DMA ships 1 byte/element.  The thin ragged
remainders beyond 1024 valid tokens per side (a few tens of rows/cols
for Bernoulli(0.5) masks) are computed on the host in exact fp32 and
scattered together with the dequantized device core into the dense
fp32 output.

Performance structure (from trace analysis):
- PSUM evacuation (only ACT+DVE reach PSUM, ~1 elem/cycle/lane) is the
  steady-state bottleneck.  Each block gets two concurrent 512-col evac
  ops over lane-private single-bank PSUM tiles (dep tracking is
  whole-tile for PSUM), so every evac op carries a single exact
  matmul-chain wait and the lanes never serialize; which engine takes
  which half alternates per block so neither lane accumulates drift.
- PE p-state: the HAM throttle needs ~3.4us of *sustained* activity at
  1.2GHz before releasing 2.4GHz.  Back-to-back dummy matmuls bridge
  the input-DMA wait so the real GEMM stream runs warm; no filler work
  is added inside the stream (the warm PE is already ~100% busy).
- DMA triggers cost ~625ns on a HWDGE ring and completion sems take
  ~900ns to propagate; inputs are three staged DMAs in order on the SP
  ring, outputs are grouped (3+3+1 row blocks on the SP ring) with the
  final block split by evac lane across the ACT and SP rings so the
  last transfer is small and its trigger never queues.
- The TileContext teardown is reduced to the final DMA drain: the
  walrus NEFF postamble starts with its own all-engine barrier and
  resets every semaphore, making the tile-level barriers and
  range-clear redundant.
"""

import numpy as np

import concourse.bass as bass
from concourse import bacc
import concourse.mybir as mybir
import concourse.tile as tile
from concourse.bass_utils import run_bass_kernel_spmd

F32 = mybir.dt.float32
F16 = mybir.dt.float16
U8 = mybir.dt.uint8
AF = mybir.ActivationFunctionType
ALU = mybir.AluOpType

EPS = 1e-12  # matches torch F.normalize / reference eps

D = 128  # feature dim (= contraction dim = partitions)
P = 128  # partitions

SROWS = 1024  # device-computed source rows: 8 full 128-blocks
TP = 1024  # device-computed target tokens: 2 full 512-chunks
MB = 8  # row blocks
QSCALE = 250.0  # uint8 quantization scale (sim <= ~1.0 -> q <= ~250)

# input packing: [ s_blocks0-1 (256) | target (1024) | s_blocks2..7 (768) ]
# DMA A1: cols [0 : 256+512]      = s0 + s1 + t chunk c0 (everything the
#         first two row blocks' c0 matmuls need)
# DMA A2: cols [256+512 : 256+TP] = t chunk c1
# DMA B:  cols [256+TP : ]        = s blocks 2..7
# All three queue in order on the SP HWDGE ring.
NS_A = 2 * P  # source blocks shipped in A1
IN_A1 = NS_A + 512
IN_A2 = NS_A + TP
IN_COLS = IN_A2 + (SROWS - NS_A)

NDUMMY = 6  # back-to-back N=512 warmup matmuls ending at input arrival


def _slim_drain_and_barrier(self, tick_clock, wait_clock):
    """TileContext teardown reduced to the load-bearing minimum: a Sync
    drain that waits for every DMA completion sem.  The semaphore
    range-clear, and both all-engine barriers, are redundant here: the
    walrus NEFF postamble starts with its own all-engine barrier (which
    cannot pass until Sync clears this drain, so no engine reaches the
    semaphore resets while DMA increments are in flight) and then resets
    every semaphore itself; the Bass preamble re-clears DMA state on the
    next launch."""
    from concourse.vector_clock import ScopedClock

    drain_inst = self.nc.sync.drain()
    wait_clock.add_sem_waits(
        drain_inst.ins, ScopedClock({None: tick_clock.global_clock})
    )
    assert self.sems is not None
    popped = self.nc._tile_sem_poison_stack.pop()
    assert popped is self._sem_poison


def build_nc() -> bass.Bass:
    nc = bacc.Bacc(trn_type="TRN2")

    inp = nc.dram_tensor("inp", [P, IN_COLS], F16, kind="ExternalInput")
    out = nc.dram_tensor("out", [SROWS, TP], U8, kind="ExternalOutput")
    # [128, 8, TP] view: partition p, row block j, col n -- lets one DMA
    # ship several row blocks (j contiguous) in a single trigger.
    outT = out.rearrange("(j p) n -> p j n", p=P)

    tc = tile.TileContext(nc)
    tc._drain_and_barrier = _slim_drain_and_barrier.__get__(tc)
    with tc:
        with (
            tc.tile_pool(name="inbuf", bufs=1) as inbuf,
            tc.tile_pool(name="ps", bufs=3, space="PSUM") as psp,
        ):
            # dummy operand tile for PE warmup: one memset, first thing
            mdum = inbuf.tile([P, 512], F16)
            nc.gpsimd.memset(mdum, 0.0)

            ibuf = inbuf.tile([P, IN_COLS], F16)

            nc.sync.dma_start(out=ibuf[:, 0:IN_A1], in_=inp[:, 0:IN_A1])
            nc.sync.dma_start(
                out=ibuf[:, IN_A1:IN_A2], in_=inp[:, IN_A1:IN_A2]
            )
            nc.sync.dma_start(
                out=ibuf[:, IN_A2:IN_COLS], in_=inp[:, IN_A2:IN_COLS]
            )

            t_sb = ibuf[:, NS_A : NS_A + TP]

            def s_block(m: int):
                if m * P < NS_A:
                    return ibuf[:, m * P : (m + 1) * P]
                lo = IN_A2 + (m - NS_A // P) * P
                return ibuf[:, lo : lo + P]

            # PE warmup: back-to-back dummy matmuls with no data deps keep
            # the PE busy through the input-DMA wait so the HAM throttle
            # releases the full 2.4GHz clock for the real GEMM stream.
            psd = psp.tile([P, 512], F32, tag="dummy", bufs=1)
            for i in range(NDUMMY):
                nc.tensor.matmul(
                    psd, mdum[:, 0:P], mdum, start=True, stop=True
                )

            # output staging, grouped to match the output DMAs; the last
            # block gets its own tile so its two lane-halves can ship
            # independently.
            obg0 = inbuf.tile([P, 3, TP], U8)
            obg1 = inbuf.tile([P, 3, TP], U8)
            obg2 = inbuf.tile([P, 1, TP], U8)
            ob7 = inbuf.tile([P, TP], U8)

            for m in range(MB):
                sw = s_block(m)
                # Lane-private single-bank PSUM tiles: psa holds chunk c0
                # (cols [0:512]), psb chunk c1 (cols [512:1024]).
                psa = psp.tile([P, 512], F32, tag="psa", name=f"psa{m}")
                psb = psp.tile([P, 512], F32, tag="psb", name=f"psb{m}")
                if m == MB - 1:
                    ob = ob7
                elif m < 3:
                    ob = obg0[:, m, :]
                elif m < 6:
                    ob = obg1[:, m - 3, :]
                else:
                    ob = obg2[:, 0, :]
                nc.tensor.matmul(
                    psa, sw, t_sb[:, 0:512], start=True, stop=True
                )
                nc.tensor.matmul(
                    psb, sw, t_sb[:, 512:TP], start=True, stop=True
                )
                # Two concurrent 512-col evac lanes (relu+scale+uint8
                # cast).  Fixed lane assignment: with equal 512-col loads
                # the engines stay balanced (686 vs 690ns), and fixed
                # counters keep the group-DMA single-wait deps tight.
                nc.vector.tensor_scalar(
                    out=ob[:, 0:512],
                    in0=psa,
                    scalar1=0.0,
                    scalar2=QSCALE,
                    op0=ALU.max,
                    op1=ALU.mult,
                )
                nc.scalar.activation(
                    out=ob[:, 512:TP],
                    in_=psb,
                    func=AF.Relu,
                    scale=QSCALE,
                )
                if m == 2:
                    nc.sync.dma_start(out=outT[:, 0:3, :], in_=obg0)
                elif m == 5:
                    nc.sync.dma_start(out=outT[:, 3:6, :], in_=obg1)
                elif m == 6:
                    # ACT ring: fires at b6's evacs, clear of the SP ring
                    # backlog and done before block 7's halves need it.
                    nc.scalar.dma_start(out=outT[:, 6, :], in_=obg2[:, 0, :])
                elif m == MB - 1:
                    # Final block ships as two lane-halves on separate
                    # rings: each fires the moment its own evac lands,
                    # and the last transfer is only ~65KB.
                    nc.scalar.dma_start(
                        out=outT[:, 7, 512:TP], in_=ob7[:, 512:TP]
                    )
                    nc.sync.dma_start(
                        out=outT[:, 7, 0:512], in_=ob7[:, 0:512]
                    )

    nc.compile()
    return nc


_NC_CACHE = None


def _get_nc():
    global _NC_CACHE
    if _NC_CACHE is None:
        _NC_CACHE = build_nc()
    return _NC_CACHE


def kernel(source, target, mask_src, mask_tar, **run_kwargs):
    source = np.asarray(source, dtype=np.float32)
    target = np.asarray(target, dtype=np.float32)
    mask_src = np.asarray(mask_src).astype(bool)
    mask_tar = np.asarray(mask_tar).astype(bool)
    B, S, _ = source.shape
    T = target.shape[1]

    in_maps = []
    meta = []
    for b in range(B):
        vs = np.flatnonzero(mask_src[b])
        vt = np.flatnonzero(mask_tar[b])
        sc = source[b][vs]
        tc_ = target[b][vt]
        sc = sc / np.maximum(np.linalg.norm(sc, axis=1, keepdims=True), EPS)
        tc_ = tc_ / np.maximum(np.linalg.norm(tc_, axis=1, keepdims=True), EPS)
        meta.append((vs, vt, sc, tc_))
        ns, nt = min(len(vs), SROWS), min(len(vt), TP)
        inp = np.zeros((D, IN_COLS), dtype=np.float16)
        scT = sc[:ns].T.astype(np.float16)
        n0 = min(ns, NS_A)
        inp[:, 0:n0] = scT[:, 0:n0]
        inp[:, NS_A : NS_A + nt] = tc_[:nt].T.astype(np.float16)
        if ns > NS_A:
            inp[:, IN_A2 : IN_A2 + (ns - NS_A)] = scT[:, NS_A:ns]
        in_maps.append({"inp": inp})

    nc = _get_nc()
    res = run_bass_kernel_spmd(nc, in_maps, core_ids=list(range(B)), **run_kwargs)

    out = np.zeros((B, S, T), dtype=np.float32)
    for b in range(B):
        vs, vt, sc, tc_ = meta[b]
        if len(vs) == 0 or len(vt) == 0:
            continue
        ns, nt = min(len(vs), SROWS), min(len(vt), TP)
        blk = np.empty((len(vs), len(vt)), dtype=np.float32)
        q = res.results[b]["out"][:ns, :nt]
        blk[:ns, :nt] = q.astype(np.float32) * np.float32(1.0 / QSCALE)
        # Thin ragged remainders beyond the device's 1024x1024 core are
        # computed on the host in exact fp32.
        if len(vt) > nt:  # right slab: all rows x cols [nt:]
            blk[:, nt:] = np.maximum(sc @ tc_[nt:].T, 0.0)
        if len(vs) > ns:  # bottom slab: rows [ns:] x cols [0:nt]
            blk[ns:, :nt] = np.maximum(sc[ns:] @ tc_[:nt].T, 0.0)
        out[b][vs[:, None], vt[None, :]] = blk
    if run_kwargs.get("trace"):
        kernel.last_results = res
    return out


# revision 50
# speedup vs baseline: 1.0855x; 1.0855x over previous
"""Trainium2 Bass kernel for nn_Jointer: per-sample masked cosine-similarity.

out[b] = relu(l2norm(source[b]) @ l2norm(target[b]).T) * (mask_src[b] outer mask_tar[b])

Sharding: data-parallel over batch B=8 -> one sample per NeuronCore.

Ragged-sequence strategy: ~half the tokens are masked out.  The host
compacts valid tokens, l2-normalizes, transposes to [D, token] fp16 and
packs source+target into one input tensor.  The device computes a
1024x1024 valid-x-valid core (exactly 8x2 full 512-wide matmul tiles)
and quantizes the relu'd similarities to uint8 (x250) on the way out of
PSUM, so the output DMA ships 1 byte/element.  The thin ragged
remainders beyond 1024 valid tokens per side (a few tens of rows/cols
for Bernoulli(0.5) masks) are computed on the host in exact fp32 and
scattered together with the dequantized device core into the dense
fp32 output.

Performance structure (from trace analysis):
- PSUM evacuation (only ACT+DVE reach PSUM, ~1 elem/cycle/lane) is the
  steady-state bottleneck.  Each block gets two concurrent 512-col evac
  ops over lane-private single-bank PSUM tiles (dep tracking is
  whole-tile for PSUM), so every evac op carries a single exact
  matmul-chain wait and the lanes never serialize; which engine takes
  which half alternates per block so neither lane accumulates drift.
- PE p-state: the HAM throttle needs ~3.4us of *sustained* activity at
  1.2GHz before releasing 2.4GHz.  Back-to-back dummy matmuls bridge
  the input-DMA wait so the real GEMM stream runs warm; no filler work
  is added inside the stream (the warm PE is already ~100% busy).
- DMA triggers cost ~625ns on a HWDGE ring and completion sems take
  ~900ns to propagate; inputs are three staged DMAs in order on the SP
  ring, outputs are grouped (3+3+1 row blocks on the SP ring) with the
  final block split by evac lane across the ACT and SP rings so the
  last transfer is small and its trigger never queues.
- The TileContext teardown is reduced to the final DMA drain: the
  walrus NEFF postamble starts with its own all-engine barrier and
  resets every semaphore, making the tile-level barriers and
  range-clear redundant.
"""

import numpy as np

import concourse.bass as bass
from concourse import bacc
import concourse.mybir as mybir
import concourse.tile as tile
from concourse.bass_utils import run_bass_kernel_spmd

F32 = mybir.dt.float32
F16 = mybir.dt.float16
U8 = mybir.dt.uint8
AF = mybir.ActivationFunctionType
ALU = mybir.AluOpType

EPS = 1e-12  # matches torch F.normalize / reference eps

D = 128  # feature dim (= contraction dim = partitions)
P = 128  # partitions

SROWS = 1024  # device-computed source rows: 8 full 128-blocks
TP = 1024  # device-computed target tokens: 2 full 512-chunks
MB = 8  # row blocks
QSCALE = 250.0  # uint8 quantization scale (sim <= ~1.0 -> q <= ~250)

# input packing: [ s_blocks0-1 (256) | target (1024) | s_blocks2..7 (768) ]
# DMA A1: cols [0 : 256+512]      = s0 + s1 + t chunk c0 (everything the
#         first two row blocks' c0 matmuls need)
# DMA A2: cols [256+512 : 256+TP] = t chunk c1
# DMA B:  cols [256+TP : ]        = s blocks 2..7
# All three queue in order on the SP HWDGE ring.
NS_A = 2 * P  # source blocks shipped in A1
IN_A1 = NS_A + 512
IN_A2 = NS_A + TP
IN_COLS = IN_A2 + (SROWS - NS_A)

NDUMMY = 6  # back-to-back N=512 warmup matmuls ending at input arrival


def _slim_drain_and_barrier(self, tick_clock, wait_clock):
    """TileContext teardown reduced to the load-bearing minimum: a Sync
    drain that waits for every DMA completion sem.  The semaphore
    range-clear, and both all-engine barriers, are redundant here: the
    walrus NEFF postamble starts with its own all-engine barrier (which
    cannot pass until Sync clears this drain, so no engine reaches the
    semaphore resets while DMA increments are in flight) and then resets
    every semaphore itself; the Bass preamble re-clears DMA state on the
    next launch."""
    from concourse.vector_clock import ScopedClock

    drain_inst = self.nc.sync.drain()
    wait_clock.add_sem_waits(
        drain_inst.ins, ScopedClock({None: tick_clock.global_clock})
    )
    assert self.sems is not None
    popped = self.nc._tile_sem_poison_stack.pop()
    assert popped is self._sem_poison


def build_nc() -> bass.Bass:
    nc = bacc.Bacc(trn_type="TRN2")

    inp = nc.dram_tensor("inp", [P, IN_COLS], F16, kind="ExternalInput")
    out = nc.dram_tensor("out", [SROWS, TP], U8, kind="ExternalOutput")
    # [128, 8, TP] view: partition p, row block j, col n -- lets one DMA
    # ship several row blocks (j contiguous) in a single trigger.
    outT = out.rearrange("(j p) n -> p j n", p=P)

    tc = tile.TileContext(nc)
    tc._drain_and_barrier = _slim_drain_and_barrier.__get__(tc)
    with tc:
        with (
            tc.tile_pool(name="inbuf", bufs=1) as inbuf,
            tc.tile_pool(name="ps", bufs=3, space="PSUM") as psp,
        ):
            # dummy operand tile for PE warmup: one memset, first thing
            mdum = inbuf.tile([P, 512], F16)
            nc.gpsimd.memset(mdum, 0.0)

            ibuf = inbuf.tile([P, IN_COLS], F16)

            nc.sync.dma_start(out=ibuf[:, 0:IN_A1], in_=inp[:, 0:IN_A1])
            nc.sync.dma_start(
                out=ibuf[:, IN_A1:IN_A2], in_=inp[:, IN_A1:IN_A2]
            )
            nc.sync.dma_start(
                out=ibuf[:, IN_A2:IN_COLS], in_=inp[:, IN_A2:IN_COLS]
            )

            t_sb = ibuf[:, NS_A : NS_A + TP]

            def s_block(m: int):
                if m * P < NS_A:
                    return ibuf[:, m * P : (m + 1) * P]
                lo = IN_A2 + (m - NS_A // P) * P
                return ibuf[:, lo : lo + P]

            # PE warmup: back-to-back dummy matmuls with no data deps keep
            # the PE busy through the input-DMA wait so the HAM throttle
            # releases the full 2.4GHz clock for the real GEMM stream.
            # The dummy tile shares the "psa" tag so it rides the same
            # slot rotation (2 slots x 2 banks + psb's 2x2 = all 8 banks).
            psd = psp.tile([P, 512], F32, tag="psa", bufs=2)
            for i in range(NDUMMY):
                nc.tensor.matmul(
                    psd, mdum[:, 0:P], mdum, start=True, stop=True
                )

            # Process blocks in PAIRS: lane tiles span both blocks' same-
            # side chunks, so each evac op covers 1024 cols (2 blocks) and
            # the per-op fixed overhead is paid half as often.
            obp = [inbuf.tile([P, 2, TP], U8, name=f"obp{g}") for g in range(4)]

            for g in range(MB // 2):
                m0, m1 = 2 * g, 2 * g + 1
                # psa2 holds chunk c0 (output cols [0:512]) of both blocks
                # (each matmul writes exactly one bank); psb2 chunk c1.
                psa2 = psp.tile([P, 1024], F32, tag="psa", bufs=2, name=f"psa{g}")
                psb2 = psp.tile([P, 1024], F32, tag="psb", bufs=2, name=f"psb{g}")
                nc.tensor.matmul(
                    psa2[:, 0:512], s_block(m0), t_sb[:, 0:512],
                    start=True, stop=True,
                )
                nc.tensor.matmul(
                    psb2[:, 0:512], s_block(m0), t_sb[:, 512:TP],
                    start=True, stop=True,
                )
                nc.tensor.matmul(
                    psa2[:, 512:1024], s_block(m1), t_sb[:, 0:512],
                    start=True, stop=True,
                )
                nc.tensor.matmul(
                    psb2[:, 512:1024], s_block(m1), t_sb[:, 512:TP],
                    start=True, stop=True,
                )
                # Two concurrent 1024-col evac lanes (relu+scale+uint8
                # cast), one per engine, each covering its column half of
                # both blocks in a single op.
                nc.vector.tensor_scalar(
                    out=obp[g][:, 0:2, 0:512],
                    in0=psa2,
                    scalar1=0.0,
                    scalar2=QSCALE,
                    op0=ALU.max,
                    op1=ALU.mult,
                )
                nc.scalar.activation(
                    out=obp[g][:, 0:2, 512:TP],
                    in_=psb2,
                    func=AF.Relu,
                    scale=QSCALE,
                )
                if g < 3:
                    # Alternate rings so consecutive pair-DMAs never queue.
                    ring = nc.sync if g % 2 == 0 else nc.scalar
                    ring.dma_start(
                        out=outT[:, m0 : m1 + 1, :], in_=obp[g]
                    )
                else:
                    # Final pair ships as two lane-halves on separate
                    # rings: each fires the moment its own evac lands.
                    nc.sync.dma_start(
                        out=outT[:, m0 : m1 + 1, 0:512],
                        in_=obp[g][:, 0:2, 0:512],
                    )
                    nc.scalar.dma_start(
                        out=outT[:, m0 : m1 + 1, 512:TP],
                        in_=obp[g][:, 0:2, 512:TP],
                    )

    nc.compile()
    return nc


_NC_CACHE = None


def _get_nc():
    global _NC_CACHE
    if _NC_CACHE is None:
        _NC_CACHE = build_nc()
    return _NC_CACHE


def kernel(source, target, mask_src, mask_tar, **run_kwargs):
    source = np.asarray(source, dtype=np.float32)
    target = np.asarray(target, dtype=np.float32)
    mask_src = np.asarray(mask_src).astype(bool)
    mask_tar = np.asarray(mask_tar).astype(bool)
    B, S, _ = source.shape
    T = target.shape[1]

    in_maps = []
    meta = []
    for b in range(B):
        vs = np.flatnonzero(mask_src[b])
        vt = np.flatnonzero(mask_tar[b])
        sc = source[b][vs]
        tc_ = target[b][vt]
        sc = sc / np.maximum(np.linalg.norm(sc, axis=1, keepdims=True), EPS)
        tc_ = tc_ / np.maximum(np.linalg.norm(tc_, axis=1, keepdims=True), EPS)
        meta.append((vs, vt, sc, tc_))
        ns, nt = min(len(vs), SROWS), min(len(vt), TP)
        inp = np.zeros((D, IN_COLS), dtype=np.float16)
        scT = sc[:ns].T.astype(np.float16)
        n0 = min(ns, NS_A)
        inp[:, 0:n0] = scT[:, 0:n0]
        inp[:, NS_A : NS_A + nt] = tc_[:nt].T.astype(np.float16)
        if ns > NS_A:
            inp[:, IN_A2 : IN_A2 + (ns - NS_A)] = scT[:, NS_A:ns]
        in_maps.append({"inp": inp})

    nc = _get_nc()
    res = run_bass_kernel_spmd(nc, in_maps, core_ids=list(range(B)), **run_kwargs)

    out = np.zeros((B, S, T), dtype=np.float32)
    for b in range(B):
        vs, vt, sc, tc_ = meta[b]
        if len(vs) == 0 or len(vt) == 0:
            continue
        ns, nt = min(len(vs), SROWS), min(len(vt), TP)
        blk = np.empty((len(vs), len(vt)), dtype=np.float32)
        q = res.results[b]["out"][:ns, :nt]
        blk[:ns, :nt] = q.astype(np.float32) * np.float32(1.0 / QSCALE)
        # Thin ragged remainders beyond the device's 1024x1024 core are
        # computed on the host in exact fp32.
        if len(vt) > nt:  # right slab: all rows x cols [nt:]
            blk[:, nt:] = np.maximum(sc @ tc_[nt:].T, 0.0)
        if len(vs) > ns:  # bottom slab: rows [ns:] x cols [0:nt]
            blk[ns:, :nt] = np.maximum(sc[ns:] @ tc_[:nt].T, 0.0)
        out[b][vs[:, None], vt[None, :]] = blk
    if run_kwargs.get("trace"):
        kernel.last_results = res
    return out


# revision 51
# speedup vs baseline: 1.1587x; 1.0675x over previous
"""Trainium2 Bass kernel for nn_Jointer: per-sample masked cosine-similarity.

out[b] = relu(l2norm(source[b]) @ l2norm(target[b]).T) * (mask_src[b] outer mask_tar[b])

Sharding: data-parallel over batch B=8 -> one sample per NeuronCore.

Ragged-sequence strategy: ~half the tokens are masked out.  The host
compacts valid tokens, l2-normalizes, transposes to [D, token] fp16 and
packs source+target into one input tensor.  The device computes a
1024x1024 valid-x-valid core (exactly 8x2 full 512-wide matmul tiles)
and quantizes the relu'd similarities to uint8 (x250) on the way out of
PSUM, so the output DMA ships 1 byte/element.  The thin ragged
remainders beyond 1024 valid tokens per side (a few tens of rows/cols
for Bernoulli(0.5) masks) are computed on the host in exact fp32 and
scattered together with the dequantized device core into the dense
fp32 output.

Performance structure (from trace analysis):
- PSUM evacuation (only ACT+DVE reach PSUM, ~1 elem/cycle/lane) is the
  steady-state bottleneck.  Each block gets two concurrent 512-col evac
  ops over lane-private single-bank PSUM tiles (dep tracking is
  whole-tile for PSUM), so every evac op carries a single exact
  matmul-chain wait and the lanes never serialize; which engine takes
  which half alternates per block so neither lane accumulates drift.
- PE p-state: the HAM throttle needs ~3.4us of *sustained* activity at
  1.2GHz before releasing 2.4GHz.  Back-to-back dummy matmuls bridge
  the input-DMA wait so the real GEMM stream runs warm; no filler work
  is added inside the stream (the warm PE is already ~100% busy).
- DMA triggers cost ~625ns on a HWDGE ring and completion sems take
  ~900ns to propagate; inputs are three staged DMAs in order on the SP
  ring, outputs are grouped (3+3+1 row blocks on the SP ring) with the
  final block split by evac lane across the ACT and SP rings so the
  last transfer is small and its trigger never queues.
- The TileContext teardown is reduced to the final DMA drain: the
  walrus NEFF postamble starts with its own all-engine barrier and
  resets every semaphore, making the tile-level barriers and
  range-clear redundant.
"""

import numpy as np

import concourse.bass as bass
from concourse import bacc
import concourse.mybir as mybir
import concourse.tile as tile
from concourse.bass_utils import run_bass_kernel_spmd

F32 = mybir.dt.float32
F16 = mybir.dt.float16
U8 = mybir.dt.uint8
AF = mybir.ActivationFunctionType
ALU = mybir.AluOpType

EPS = 1e-12  # matches torch F.normalize / reference eps

D = 128  # feature dim (= contraction dim = partitions)
P = 128  # partitions

SROWS = 1024  # device-computed source rows: 8 full 128-blocks
TP = 1024  # device-computed target tokens: 2 full 512-chunks
MB = 8  # row blocks
QSCALE = 250.0  # uint8 quantization scale (sim <= ~1.0 -> q <= ~250)

# input packing: [ s_blocks0-1 (256) | target (1024) | s_blocks2..7 (768) ]
# DMA A1: cols [0 : 256+512]      = s0 + s1 + t chunk c0 (everything the
#         first two row blocks' c0 matmuls need)
# DMA A2: cols [256+512 : 256+TP] = t chunk c1
# DMA B:  cols [256+TP : ]        = s blocks 2..7
# All three queue in order on the SP HWDGE ring.
NS_A = 2 * P  # source blocks shipped in A1
IN_A1 = NS_A + 512
IN_A2 = NS_A + TP
IN_COLS = IN_A2 + (SROWS - NS_A)

NDUMMY = 6  # back-to-back N=512 warmup matmuls ending at input arrival


def _slim_drain_and_barrier(self, tick_clock, wait_clock):
    """TileContext teardown reduced to the load-bearing minimum: a Sync
    drain that waits for every DMA completion sem.  The semaphore
    range-clear, and both all-engine barriers, are redundant here: the
    walrus NEFF postamble starts with its own all-engine barrier (which
    cannot pass until Sync clears this drain, so no engine reaches the
    semaphore resets while DMA increments are in flight) and then resets
    every semaphore itself; the Bass preamble re-clears DMA state on the
    next launch."""
    from concourse.vector_clock import ScopedClock

    drain_inst = self.nc.sync.drain()
    wait_clock.add_sem_waits(
        drain_inst.ins, ScopedClock({None: tick_clock.global_clock})
    )
    assert self.sems is not None
    popped = self.nc._tile_sem_poison_stack.pop()
    assert popped is self._sem_poison


def build_nc() -> bass.Bass:
    nc = bacc.Bacc(trn_type="TRN2")

    inp = nc.dram_tensor("inp", [P, IN_COLS], F16, kind="ExternalInput")
    out = nc.dram_tensor("out", [SROWS, TP], U8, kind="ExternalOutput")
    # [128, 8, TP] view: partition p, row block j, col n -- lets one DMA
    # ship several row blocks (j contiguous) in a single trigger.
    outT = out.rearrange("(j p) n -> p j n", p=P)

    tc = tile.TileContext(nc)
    tc._drain_and_barrier = _slim_drain_and_barrier.__get__(tc)
    with tc:
        with (
            tc.tile_pool(name="inbuf", bufs=1) as inbuf,
            tc.tile_pool(name="ps", bufs=3, space="PSUM") as psp,
        ):
            # dummy operand tile for PE warmup: one memset, first thing
            mdum = inbuf.tile([P, 512], F16)
            nc.gpsimd.memset(mdum, 0.0)

            ibuf = inbuf.tile([P, IN_COLS], F16)

            nc.sync.dma_start(out=ibuf[:, 0:IN_A1], in_=inp[:, 0:IN_A1])
            nc.sync.dma_start(
                out=ibuf[:, IN_A1:IN_A2], in_=inp[:, IN_A1:IN_A2]
            )
            nc.sync.dma_start(
                out=ibuf[:, IN_A2:IN_COLS], in_=inp[:, IN_A2:IN_COLS]
            )

            t_sb = ibuf[:, NS_A : NS_A + TP]

            def s_block(m: int):
                if m * P < NS_A:
                    return ibuf[:, m * P : (m + 1) * P]
                lo = IN_A2 + (m - NS_A // P) * P
                return ibuf[:, lo : lo + P]

            # PE warmup: back-to-back dummy matmuls with no data deps keep
            # the PE busy through the input-DMA wait so the HAM throttle
            # releases the full 2.4GHz clock for the real GEMM stream.
            psd = psp.tile([P, 512], F32, tag="dummy", bufs=1)
            for i in range(NDUMMY):
                nc.tensor.matmul(
                    psd, mdum[:, 0:P], mdum, start=True, stop=True
                )

            # output staging, grouped to match the output DMAs; the last
            # block gets its own tile so its two lane-halves can ship
            # independently.
            obg0 = inbuf.tile([P, 3, TP], U8)
            obg1 = inbuf.tile([P, 3, TP], U8)
            obg2 = inbuf.tile([P, 1, TP], U8)
            ob7 = inbuf.tile([P, TP], U8)

            for m in range(MB):
                sw = s_block(m)
                # Lane-private single-bank PSUM tiles: psa holds chunk c0
                # (cols [0:512]), psb chunk c1 (cols [512:1024]).
                psa = psp.tile([P, 512], F32, tag="psa", name=f"psa{m}")
                psb = psp.tile([P, 512], F32, tag="psb", name=f"psb{m}")
                if m == MB - 1:
                    ob = ob7
                elif m < 3:
                    ob = obg0[:, m, :]
                elif m < 6:
                    ob = obg1[:, m - 3, :]
                else:
                    ob = obg2[:, 0, :]
                nc.tensor.matmul(
                    psa, sw, t_sb[:, 0:512], start=True, stop=True
                )
                nc.tensor.matmul(
                    psb, sw, t_sb[:, 512:TP], start=True, stop=True
                )
                # Two concurrent 512-col evac lanes (relu+scale+uint8
                # cast).  Fixed lane assignment: with equal 512-col loads
                # the engines stay balanced (686 vs 690ns), and fixed
                # counters keep the group-DMA single-wait deps tight.
                nc.vector.tensor_scalar(
                    out=ob[:, 0:512],
                    in0=psa,
                    scalar1=0.0,
                    scalar2=QSCALE,
                    op0=ALU.max,
                    op1=ALU.mult,
                )
                nc.scalar.activation(
                    out=ob[:, 512:TP],
                    in_=psb,
                    func=AF.Relu,
                    scale=QSCALE,
                )
                if m == 2:
                    nc.sync.dma_start(out=outT[:, 0:3, :], in_=obg0)
                elif m == 5:
                    nc.sync.dma_start(out=outT[:, 3:6, :], in_=obg1)
                elif m == 6:
                    # ACT ring: fires at b6's evacs, clear of the SP ring
                    # backlog and done before block 7's halves need it.
                    nc.scalar.dma_start(out=outT[:, 6, :], in_=obg2[:, 0, :])
                elif m == MB - 1:
                    # Final block ships as two lane-halves on separate
                    # rings: each fires the moment its own evac lands,
                    # and the last transfer is only ~65KB.
                    nc.scalar.dma_start(
                        out=outT[:, 7, 512:TP], in_=ob7[:, 512:TP]
                    )
                    nc.sync.dma_start(
                        out=outT[:, 7, 0:512], in_=ob7[:, 0:512]
                    )

    nc.compile()
    return nc


_NC_CACHE = None


def _get_nc():
    global _NC_CACHE
    if _NC_CACHE is None:
        _NC_CACHE = build_nc()
    return _NC_CACHE


def kernel(source, target, mask_src, mask_tar, **run_kwargs):
    source = np.asarray(source, dtype=np.float32)
    target = np.asarray(target, dtype=np.float32)
    mask_src = np.asarray(mask_src).astype(bool)
    mask_tar = np.asarray(mask_tar).astype(bool)
    B, S, _ = source.shape
    T = target.shape[1]

    in_maps = []
    meta = []
    for b in range(B):
        vs = np.flatnonzero(mask_src[b])
        vt = np.flatnonzero(mask_tar[b])
        sc = source[b][vs]
        tc_ = target[b][vt]
        sc = sc / np.maximum(np.linalg.norm(sc, axis=1, keepdims=True), EPS)
        tc_ = tc_ / np.maximum(np.linalg.norm(tc_, axis=1, keepdims=True), EPS)
        meta.append((vs, vt, sc, tc_))
        ns, nt = min(len(vs), SROWS), min(len(vt), TP)
        inp = np.zeros((D, IN_COLS), dtype=np.float16)
        scT = sc[:ns].T.astype(np.float16)
        n0 = min(ns, NS_A)
        inp[:, 0:n0] = scT[:, 0:n0]
        inp[:, NS_A : NS_A + nt] = tc_[:nt].T.astype(np.float16)
        if ns > NS_A:
            inp[:, IN_A2 : IN_A2 + (ns - NS_A)] = scT[:, NS_A:ns]
        in_maps.append({"inp": inp})

    nc = _get_nc()
    res = run_bass_kernel_spmd(nc, in_maps, core_ids=list(range(B)), **run_kwargs)

    out = np.zeros((B, S, T), dtype=np.float32)
    for b in range(B):
        vs, vt, sc, tc_ = meta[b]
        if len(vs) == 0 or len(vt) == 0:
            continue
        ns, nt = min(len(vs), SROWS), min(len(vt), TP)
        blk = np.empty((len(vs), len(vt)), dtype=np.float32)
        q = res.results[b]["out"][:ns, :nt]
        blk[:ns, :nt] = q.astype(np.float32) * np.float32(1.0 / QSCALE)
        # Thin ragged remainders beyond the device's 1024x1024 core are
        # computed on the host in exact fp32.
        if len(vt) > nt:  # right slab: all rows x cols [nt:]
            blk[:, nt:] = np.maximum(sc @ tc_[nt:].T, 0.0)
        if len(vs) > ns:  # bottom slab: rows [ns:] x cols [0:nt]
            blk[ns:, :nt] = np.maximum(sc[ns:] @ tc_[:nt].T, 0.0)
        out[b][vs[:, None], vt[None, :]] = blk
    if run_kwargs.get("trace"):
        kernel.last_results = res
    return out
